# revision 33
# baseline (speedup 1.0000x reference)
"""Multi-head causal attention (B=4, S=2048, E=1024, H=16, D=64) on 8 TRN2
NeuronCores.

Sharding: core c handles batch b = c//2 and head-group g = c%2 (8 heads).
Per core: QKV projections, causal attention, partial output projection over
its 512 input dims of Wproj (+ bproj/2), then a pairwise ReduceScatter sums
the two head-group partials; core 2b returns rows 0:1024 of batch b, core
2b+1 rows 1024:2048.

Layout choices:
 - x is passed pre-transposed per batch: xT [E, S] so it serves directly as
   matmul operands (contraction on partitions).
 - q/k are computed transposed per head-pair: qT/kT [128, S] with rows
   0:64 = head 2c dims, 64:128 = head 2c+1 dims. Scores are computed
   TRANSPOSED (scoresT[k, q]) so that the AV matmul can consume attn with k
   on partitions; the two heads of a pair run concurrently on the PE via
   64-row tile packing.
 - v is stored per k-chunk as [128, 130]: two 65-wide head slots (64 v dims
   + a ones column). The ones column makes the AV matmul emit the softmax
   denominator as output row 64 (no max-subtraction needed: logits are
   O(10), exp is safe in f32).
 - Causal masking: block-granular skipping plus a post-exp multiply by a
   0/1 mask on diagonal blocks only.
"""

import numpy as np
import ml_dtypes

import concourse.bass as bass
import concourse.mybir as mybir
import concourse.bacc as bacc
import concourse.tile as tile
from concourse.bass_utils import run_bass_kernel_spmd

F32 = mybir.dt.float32
BF16 = mybir.dt.bfloat16
EXP = mybir.ActivationFunctionType.Exp

B, S, E, H, D = 4, 2048, 1024, 16, 64
HL = H // 2          # heads per core = 8
NPAIR = HL // 2      # head pairs per core = 4
NEC = E // 128       # e-chunks = 8
NKC = S // 128       # k-chunks = 16
NQB = S // 512       # q blocks = 4
NQC = S // 128       # q chunks = 16
N_CORES = 8

_compiled_nc = None


def _build_nc():
    nc = bacc.Bacc("TRN2", target_bir_lowering=False, debug=False,
                   num_devices=N_CORES)

    xT = nc.dram_tensor("xT", [E, S], BF16, kind="ExternalInput")
    # [qk, pair, 128e-within-chunk, echunk, 128(jh*64+d)]
    wqk = nc.dram_tensor("wqk", [2, NPAIR, 128, NEC, 128], BF16,
                         kind="ExternalInput")
    wv = nc.dram_tensor("wv", [NPAIR, 128, NEC, 128], BF16,
                        kind="ExternalInput")
    # [128(jh*64+d within chunk), global hd-chunk, 512 own e-cols]
    wproj = nc.dram_tensor("wproj", [128, 2 * NPAIR, 512], BF16,
                           kind="ExternalInput")
    biash = nc.dram_tensor("biash", [1, 512], BF16, kind="ExternalInput")
    masks = nc.dram_tensor("masks", [128, 4, 512], BF16, kind="ExternalInput")
    out = nc.dram_tensor("out", [S, 512], F32, kind="ExternalOutput")

    with tile.TileContext(nc) as tc:
        with (
            tc.tile_pool(name="persist", bufs=1) as persist,
            tc.tile_pool(name="dram", bufs=1, space="DRAM") as dram,
        ):
            xT_sb = persist.tile([128, NEC * S], BF16, tag="xT")
            mask_sb = persist.tile([128, 4 * 512], BF16, tag="masks")
            wproj_sb = persist.tile([128, 2 * NPAIR * 512], BF16,
                                    tag="wproj")
            bias_sb = persist.tile([1, 512], BF16, tag="bias")
            ones_sb = persist.tile([1, 128], BF16, tag="ones")
            concatT = persist.tile([128, NPAIR * S], BF16, tag="concatT")
            # gathered concat chunks from both cores of the pair: tile c
            # holds global hd-chunk c (cols 0:S) and chunk 4+c (cols S:2S)
            gathT = [persist.tile([128, 2 * S], BF16, tag=f"gath{c}",
                                  name=f"gath{c}")
                     for c in range(NPAIR)]

            for ec in range(NEC):
                nc.sync.dma_start(xT_sb[:, ec * S:(ec + 1) * S],
                                  xT[ec * 128:(ec + 1) * 128, :])
            nc.sync.dma_start(
                mask_sb[:].rearrange("p (d j) -> p d j", d=4), masks[:])
            nc.sync.dma_start(
                wproj_sb[:].rearrange("p (c e) -> p c e", c=2 * NPAIR),
                wproj[:])
            nc.sync.dma_start(bias_sb[:], biash[:])
            nc.vector.memset(ones_sb[:], 1.0)



            with (
                tc.tile_pool(name="pair", bufs=2) as pair_pool,
                tc.tile_pool(name="attn", bufs=2) as attn_pool,
                tc.tile_pool(name="norm", bufs=2) as norm_pool,
                tc.tile_pool(name="mult", bufs=4) as mult_pool,
                tc.tile_pool(name="psqv", bufs=2, space="PSUM") as psqv,
                tc.tile_pool(name="psS", bufs=1, space="PSUM") as psS,  # 4 banks
                tc.tile_pool(name="psAV", bufs=1, space="PSUM") as psAV,  # 2
            ):
                def qkv_steps(c):
                    """Generator: QKV compute for pair c, one PE group per
                    yield. Produces (qT, kT, v_sb) tiles via closure dict."""
                    res = {}
                    wqk_sb = pair_pool.tile([128, 2 * NEC * 128], BF16,
                                            tag="wqk")
                    wv_sb = pair_pool.tile([128, NEC * 128], BF16, tag="wv")
                    for t in range(2):
                        nc.sync.dma_start(
                            wqk_sb[:, t * 1024:(t + 1) * 1024].rearrange(
                                "p (e j) -> p e j", e=NEC),
                            wqk[t, c])
                    nc.sync.dma_start(
                        wv_sb[:].rearrange("p (e j) -> p e j", e=NEC),
                        wv[c])
                    qT = pair_pool.tile([128, S], BF16, tag="qT")
                    kT = pair_pool.tile([128, S], BF16, tag="kT")
                    v_sb = pair_pool.tile([128, NKC * 130], BF16, tag="v")
                    res["qT"], res["kT"], res["v"] = qT, kT, v_sb
                    yield res
                    for t, dst in ((0, qT), (1, kT)):
                        for nb in range(4):
                            ps = psqv.tile([128, 512], F32, tag="psqv")
                            for ec in range(NEC):
                                nc.tensor.matmul(
                                    ps[:],
                                    lhsT=wqk_sb[:, t * 1024 + ec * 128:
                                                t * 1024 + ec * 128 + 128],
                                    rhs=xT_sb[:, ec * S + nb * 512:
                                              ec * S + nb * 512 + 512],
                                    start=(ec == 0), stop=(ec == NEC - 1))
                            nc.vector.tensor_copy(
                                dst[:, nb * 512:nb * 512 + 512], ps[:])
                            yield res
                    for kc in range(NKC):
                        ps = psqv.tile([128, 512], F32, tag="psqv")
                        for ec in range(NEC):
                            nc.tensor.matmul(
                                ps[:, 0:128],
                                lhsT=xT_sb[:, ec * S + kc * 128:
                                           ec * S + kc * 128 + 128],
                                rhs=wv_sb[:, ec * 128:ec * 128 + 128],
                                start=(ec == 0), stop=(ec == NEC - 1))
                        base = kc * 130
                        nc.vector.tensor_copy(
                            v_sb[:, base:base + 130].rearrange(
                                "p (j x) -> p j x", j=2)[:, :, 0:64],
                            ps[:, 0:128].rearrange("p (j d) -> p j d", j=2))
                        nc.vector.memset(v_sb[:, base + 64:base + 65], 1.0)
                        nc.vector.memset(v_sb[:, base + 129:base + 130], 1.0)
                        yield res

                def attention(c, tiles, filler, zg_d):
                    """Attention for pair c. `filler` is a generator whose
                    steps (next pair's QKV groups) are interleaved between
                    scores and AV matmuls to keep the PE busy while ACT
                    does exp."""
                    qT, kT, v_sb = tiles["qT"], tiles["kT"], tiles["v"]
                    for qb in range(NQB):
                        nkc = 4 * qb + 4
                        avA = psAV.tile([65, 512], F32, tag="avA")
                        avB = psAV.tile([65, 512], F32, tag="avB")
                        for kcg in range(0, nkc, 2):
                            # one PSUM tile holds both heads x 2 k-chunks:
                            # [A kc, A kc+1, B kc, B kc+1] -> single exp
                            sAB = psS.tile([128, 2048], F32, tag="sAB")
                            for dk in range(2):
                                kc = kcg + dk
                                for jh, o in ((0, 0), (1, 64)):
                                    nc.tensor.matmul(
                                        sAB[:, jh * 1024 + dk * 512:
                                            jh * 1024 + dk * 512 + 512],
                                        lhsT=kT[o:o + 64,
                                                kc * 128:kc * 128 + 128],
                                        rhs=qT[o:o + 64,
                                               qb * 512:qb * 512 + 512],
                                        start=True, stop=True)
                            at = attn_pool.tile([128, 2048], BF16,
                                                tag="at")
                            nc.scalar.activation(at[:], sAB[:], EXP,
                                                 scale=0.125)
                            for dk in range(2):
                                kc = kcg + dk
                                d = kc - 4 * qb
                                if d >= 0:
                                    for jh in range(2):
                                        sl = at[:, jh * 1024 + dk * 512:
                                                jh * 1024 + dk * 512 + 512]
                                        nc.vector.tensor_mul(
                                            sl, sl,
                                            mask_sb[:, d * 512:d * 512 + 512])
                            for _ in range(2):
                                if filler is not None:
                                    if next(filler, None) is None:
                                        filler = None
                            for dk in range(2):
                                kc = kcg + dk
                                st = (kc == 0)
                                sp = (kc == nkc - 1)
                                nc.tensor.matmul(
                                    avA[:],
                                    lhsT=v_sb[:, kc * 130:kc * 130 + 65],
                                    rhs=at[:, dk * 512:dk * 512 + 512],
                                    start=st, stop=sp)
                                nc.tensor.matmul(
                                    avB[:],
                                    lhsT=v_sb[:, kc * 130 + 65:
                                              kc * 130 + 130],
                                    rhs=at[:, 1024 + dk * 512:
                                            1024 + dk * 512 + 512],
                                    start=st, stop=sp)
                        # denominators to DRAM gather buf; unnormalized
                        # outputs to concatT
                        for jh, avX in ((0, avA), (1, avB)):
                            row = jh * NQB + qb
                            zrow = norm_pool.tile([1, 512], F32, tag="zrow")
                            nc.vector.tensor_copy(zrow[:], avX[64:65, :])
                            nc.sync.dma_start(zg_d[row:row + 1, :], zrow[:])
                            nc.vector.tensor_copy(
                                concatT[jh * 64:jh * 64 + 64,
                                        c * S + qb * 512:
                                        c * S + qb * 512 + 512],
                                avX[0:64, :])
                    return filler

                def normalize(c, zg_d):
                    zgs = norm_pool.tile([HL, 512], F32, tag="zgs")
                    rzf = norm_pool.tile([HL, 512], F32, tag="rzf")
                    rzb = norm_pool.tile([HL, 512], BF16, tag="rzb")
                    rz_d = dram.tile([HL, 512], BF16, tag="rz_d")
                    nc.sync.dma_start(zgs[:], zg_d[:])
                    nc.vector.reciprocal_approx_fast(rzf[:], zgs[:])
                    nc.vector.tensor_copy(rzb[:], rzf[:])
                    nc.sync.dma_start(rz_d[:], rzb[:])
                    for qb in range(NQB):
                        mt = mult_pool.tile([128, 512], BF16, tag="mult")
                        for jh in range(2):
                            row = jh * NQB + qb
                            nc.sync.dma_start(
                                mt[jh * 64:jh * 64 + 64, :],
                                rz_d[row:row + 1, :].to_broadcast((64, 512)))
                        sl = concatT[:, c * S + qb * 512:
                                     c * S + qb * 512 + 512]
                        nc.vector.tensor_mul(sl, sl, mt[:])
                    # exchange this chunk with the pair peer while later
                    # pairs are still in attention
                    cpart_d = dram.tile([128, S], BF16, tag="cpart")
                    gath_d = dram.tile([256, S], BF16, tag="gath_d")
                    nc.sync.dma_start(cpart_d[:],
                                      concatT[:, c * S:(c + 1) * S])
                    nc.gpsimd.collective_compute(
                        "AllGather",
                        mybir.AluOpType.bypass,
                        replica_groups=[[0, 1], [2, 3], [4, 5], [6, 7]],
                        ins=[cpart_d.opt()],
                        outs=[gath_d.opt()],
                    )
                    nc.sync.dma_start(gathT[c][:, 0:S], gath_d[0:128, :])
                    nc.sync.dma_start(gathT[c][:, S:2 * S],
                                      gath_d[128:256, :])

                # --- main pipeline over head pairs ---
                gen = qkv_steps(0)
                tiles0 = next(gen)
                for _ in gen:
                    pass
                tiles = tiles0
                for c in range(NPAIR):
                    if c + 1 < NPAIR:
                        nxt_gen = qkv_steps(c + 1)
                        nxt_tiles = next(nxt_gen)
                    else:
                        nxt_gen, nxt_tiles = None, None
                    zg_d = dram.tile([HL, 512], F32, tag="zg_d")
                    leftover = attention(c, tiles, nxt_gen, zg_d)
                    if leftover is not None:
                        for _ in leftover:
                            pass
                    normalize(c, zg_d)
                    tiles = nxt_tiles

            # --- output projection over all 8 gathered hd-chunks, for this
            # core's 512 output columns, + full bias ---
            with (
                tc.tile_pool(name="psP", bufs=4, space="PSUM") as psP,
                tc.tile_pool(name="outsb", bufs=4) as outsb,
            ):
                for qc in range(NQC):
                    ps = psP.tile([128, 512], F32, tag="psP")
                    for gc in range(2 * NPAIR):
                        nc.tensor.matmul(
                            ps[:],
                            lhsT=gathT[gc % NPAIR][
                                :, (gc // NPAIR) * S + qc * 128:
                                (gc // NPAIR) * S + qc * 128 + 128],
                            rhs=wproj_sb[:, gc * 512:gc * 512 + 512],
                            start=(gc == 0), stop=False)
                    nc.tensor.matmul(
                        ps[:], lhsT=ones_sb[:], rhs=bias_sb[:],
                        start=False, stop=True)
                    ot = outsb.tile([128, 512], F32, tag="ot")
                    nc.vector.tensor_copy(ot[:], ps[:])
                    nc.sync.dma_start(
                        out[qc * 128:qc * 128 + 128, :], ot[:])

    nc.compile()
    return nc


def _get_nc():
    global _compiled_nc
    if _compiled_nc is None:
        _compiled_nc = _build_nc()
    return _compiled_nc


def _pack_heads(w):
    """[8, 1024, 64] -> [4, 128, 8, 128]: (pair, e128, echunk, jh*64+d)."""
    w = w.reshape(NPAIR, 2, NEC, 128, D)
    w = w.transpose(0, 3, 2, 1, 4)
    return np.ascontiguousarray(w.reshape(NPAIR, 128, NEC, 128))


def _build_masks():
    c2 = np.arange(512)[None, None, :]
    p = np.arange(128)[:, None, None]
    d = np.arange(4)[None, :, None]
    return (c2 >= p + 128 * d).astype(ml_dtypes.bfloat16)


def make_in_maps(x, Wq, Wk, Wv, Wproj, bproj):
    bf = ml_dtypes.bfloat16
    masks_np = _build_masks()
    x = np.asarray(x, dtype=np.float32)
    Wq, Wk, Wv = (np.asarray(a, dtype=np.float32) for a in (Wq, Wk, Wv))
    Wproj = np.asarray(Wproj, dtype=np.float32)
    bproj = np.asarray(bproj, dtype=np.float32)
    in_maps = []
    for c in range(N_CORES):
        b, g = c // 2, c % 2
        hs = slice(g * HL, (g + 1) * HL)
        in_maps.append({
            "xT": np.ascontiguousarray(x[b].T).astype(bf),
            "wqk": np.stack([_pack_heads(Wq[hs]), _pack_heads(Wk[hs])],
                            axis=0).astype(bf),
            "wv": _pack_heads(Wv[hs]).astype(bf),
            "wproj": np.ascontiguousarray(
                Wproj[:, g * 512:(g + 1) * 512]
                .reshape(2 * NPAIR, 128, 512).transpose(1, 0, 2)
            ).astype(bf),
            "biash": bproj[None, g * 512:(g + 1) * 512].astype(bf),
            "masks": masks_np,
        })
    return in_maps


def assemble(results):
    """Each core returns the full S rows for its 512 output columns."""
    out = np.empty((B, S, E), dtype=np.float32)
    for c in range(N_CORES):
        b, g = c // 2, c % 2
        out[b, :, g * 512:(g + 1) * 512] = results[c]["out"]
    return out


def kernel(x, Wq, Wk, Wv, Wproj, bproj):
    nc = _get_nc()
    in_maps = make_in_maps(x, Wq, Wk, Wv, Wproj, bproj)
    res = run_bass_kernel_spmd(nc, in_maps, list(range(N_CORES))).results
    return assemble(res)


# revision 34
# speedup vs baseline: 1.3901x; 1.3901x over previous
"""Multi-head causal attention (B=4, S=2048, E=1024, H=16, D=64) on 8 TRN2
NeuronCores.

Sharding: core c handles batch b = c//2 and head-group g = c%2 (8 heads).
Per core: QKV projections, causal attention, partial output projection over
its 512 input dims of Wproj (+ bproj/2), then a pairwise ReduceScatter sums
the two head-group partials; core 2b returns rows 0:1024 of batch b, core
2b+1 rows 1024:2048.

Layout choices:
 - x is passed pre-transposed per batch: xT [E, S] so it serves directly as
   matmul operands (contraction on partitions).
 - q/k are computed transposed per head-pair: qT/kT [128, S] with rows
   0:64 = head 2c dims, 64:128 = head 2c+1 dims. Scores are computed
   TRANSPOSED (scoresT[k, q]) so that the AV matmul can consume attn with k
   on partitions; the two heads of a pair run concurrently on the PE via
   64-row tile packing.
 - v is stored per k-chunk as [128, 130]: two 65-wide head slots (64 v dims
   + a ones column). The ones column makes the AV matmul emit the softmax
   denominator as output row 64 (no max-subtraction needed: logits are
   O(10), exp is safe in f32).
 - Causal masking: block-granular skipping plus a post-exp multiply by a
   0/1 mask on diagonal blocks only.
"""

import numpy as np
import ml_dtypes

import concourse.bass as bass
import concourse.mybir as mybir
import concourse.bacc as bacc
import concourse.tile as tile
from concourse.bass_utils import run_bass_kernel_spmd

F32 = mybir.dt.float32
BF16 = mybir.dt.bfloat16
EXP = mybir.ActivationFunctionType.Exp

B, S, E, H, D = 4, 2048, 1024, 16, 64
HL = H // 2          # heads per core = 8
NPAIR = HL // 2      # head pairs per core = 4
NEC = E // 128       # e-chunks = 8
NKC = S // 128       # k-chunks = 16
NQB = S // 512       # q blocks = 4
NQC = S // 128       # q chunks = 16
N_CORES = 8

_compiled_nc = None


def _build_nc():
    nc = bacc.Bacc("TRN2", target_bir_lowering=False, debug=False,
                   num_devices=N_CORES)

    xT = nc.dram_tensor("xT", [E, S], BF16, kind="ExternalInput")
    # [qk, pair, 128e-within-chunk, echunk, 128(jh*64+d)]
    wqk = nc.dram_tensor("wqk", [2, NPAIR, 128, NEC, 128], BF16,
                         kind="ExternalInput")
    wv = nc.dram_tensor("wv", [NPAIR, 128, NEC, 128], BF16,
                        kind="ExternalInput")
    # [128(jh*64+d within chunk), global hd-chunk, 512 own e-cols]
    wproj = nc.dram_tensor("wproj", [128, 2 * NPAIR, 512], BF16,
                           kind="ExternalInput")
    biash = nc.dram_tensor("biash", [1, 512], BF16, kind="ExternalInput")
    masks = nc.dram_tensor("masks", [128, 4, 512], BF16, kind="ExternalInput")
    out = nc.dram_tensor("out", [S, 512], F32, kind="ExternalOutput")

    with tile.TileContext(nc) as tc:
        with (
            tc.tile_pool(name="persist", bufs=1) as persist,
            tc.tile_pool(name="dram", bufs=1, space="DRAM") as dram,
        ):
            xT_sb = persist.tile([128, NEC * S], BF16, tag="xT")
            mask_sb = persist.tile([128, 4 * 512], BF16, tag="masks")
            wproj_sb = persist.tile([128, 2 * NPAIR * 512], BF16,
                                    tag="wproj")
            bias_sb = persist.tile([1, 512], BF16, tag="bias")
            ones_sb = persist.tile([1, 128], BF16, tag="ones")
            concatT = persist.tile([128, NPAIR * S], BF16, tag="concatT")
            # gathered concat chunks from both cores of the pair: tile c
            # holds global hd-chunk c (cols 0:S) and chunk 4+c (cols S:2S)
            gathT = [persist.tile([128, 2 * S], BF16, tag=f"gath{c}",
                                  name=f"gath{c}")
                     for c in range(NPAIR)]

            for ec in range(NEC):
                nc.sync.dma_start(xT_sb[:, ec * S:(ec + 1) * S],
                                  xT[ec * 128:(ec + 1) * 128, :])
            nc.sync.dma_start(
                mask_sb[:].rearrange("p (d j) -> p d j", d=4), masks[:])
            nc.sync.dma_start(
                wproj_sb[:].rearrange("p (c e) -> p c e", c=2 * NPAIR),
                wproj[:])
            nc.sync.dma_start(bias_sb[:], biash[:])
            nc.vector.memset(ones_sb[:], 1.0)



            with (
                tc.tile_pool(name="pair", bufs=2) as pair_pool,
                tc.tile_pool(name="attn", bufs=2) as attn_pool,
                tc.tile_pool(name="norm", bufs=2) as norm_pool,
                tc.tile_pool(name="mult", bufs=4) as mult_pool,
                tc.tile_pool(name="psqv", bufs=2, space="PSUM") as psqv,
                tc.tile_pool(name="psS", bufs=1, space="PSUM") as psS,  # 4 banks
                tc.tile_pool(name="psAV", bufs=1, space="PSUM") as psAV,  # 2
            ):
                def qkv_steps(c):
                    """Generator: QKV compute for pair c, one PE group per
                    yield. Produces (qT, kT, v_sb) tiles via closure dict."""
                    res = {}
                    wqk_sb = pair_pool.tile([128, 2 * NEC * 128], BF16,
                                            tag="wqk")
                    wv_sb = pair_pool.tile([128, NEC * 128], BF16, tag="wv")
                    for t in range(2):
                        nc.sync.dma_start(
                            wqk_sb[:, t * 1024:(t + 1) * 1024].rearrange(
                                "p (e j) -> p e j", e=NEC),
                            wqk[t, c])
                    nc.sync.dma_start(
                        wv_sb[:].rearrange("p (e j) -> p e j", e=NEC),
                        wv[c])
                    qT = pair_pool.tile([128, S], BF16, tag="qT")
                    kT = pair_pool.tile([128, S], BF16, tag="kT")
                    v_sb = pair_pool.tile([128, NKC * 130], BF16, tag="v")
                    res["qT"], res["kT"], res["v"] = qT, kT, v_sb
                    yield res
                    for t, dst in ((0, qT), (1, kT)):
                        for nb in range(4):
                            ps = psqv.tile([128, 512], F32, tag="psqv")
                            for ec in range(NEC):
                                nc.tensor.matmul(
                                    ps[:],
                                    lhsT=wqk_sb[:, t * 1024 + ec * 128:
                                                t * 1024 + ec * 128 + 128],
                                    rhs=xT_sb[:, ec * S + nb * 512:
                                              ec * S + nb * 512 + 512],
                                    start=(ec == 0), stop=(ec == NEC - 1))
                            nc.vector.tensor_copy(
                                dst[:, nb * 512:nb * 512 + 512], ps[:])
                            yield res
                    for kc in range(NKC):
                        ps = psqv.tile([128, 512], F32, tag="psqv")
                        for ec in range(NEC):
                            nc.tensor.matmul(
                                ps[:, 0:128],
                                lhsT=xT_sb[:, ec * S + kc * 128:
                                           ec * S + kc * 128 + 128],
                                rhs=wv_sb[:, ec * 128:ec * 128 + 128],
                                start=(ec == 0), stop=(ec == NEC - 1))
                        base = kc * 130
                        nc.vector.tensor_copy(
                            v_sb[:, base:base + 130].rearrange(
                                "p (j x) -> p j x", j=2)[:, :, 0:64],
                            ps[:, 0:128].rearrange("p (j d) -> p j d", j=2))
                        nc.vector.memset(v_sb[:, base + 64:base + 65], 1.0)
                        nc.vector.memset(v_sb[:, base + 129:base + 130], 1.0)
                        yield res

                def attention(c, tiles, filler, zg_d):
                    """Attention for pair c. `filler` is a generator whose
                    steps (next pair's QKV groups) are interleaved between
                    scores and AV matmuls to keep the PE busy while ACT
                    does exp."""
                    qT, kT, v_sb = tiles["qT"], tiles["kT"], tiles["v"]
                    for qb in range(NQB):
                        nkc = 4 * qb + 4
                        avA = psAV.tile([65, 512], F32, tag="avA")
                        avB = psAV.tile([65, 512], F32, tag="avB")
                        for kcg in range(0, nkc, 2):
                            sA = psS.tile([128, 1024], F32, tag="sA")
                            sB = psS.tile([128, 1024], F32, tag="sB")
                            for dk in range(2):
                                kc = kcg + dk
                                for jh, (sX, o) in enumerate(
                                        ((sA, 0), (sB, 64))):
                                    nc.tensor.matmul(
                                        sX[:, dk * 512:dk * 512 + 512],
                                        lhsT=kT[o:o + 64,
                                                kc * 128:kc * 128 + 128],
                                        rhs=qT[o:o + 64,
                                               qb * 512:qb * 512 + 512],
                                        start=True, stop=True)
                            atA = attn_pool.tile([128, 1024], BF16,
                                                 tag="atA")
                            atB = attn_pool.tile([128, 1024], BF16,
                                                 tag="atB")
                            nc.scalar.activation(atA[:], sA[:], EXP,
                                                 scale=0.125)
                            nc.scalar.activation(atB[:], sB[:], EXP,
                                                 scale=0.125)
                            for dk in range(2):
                                kc = kcg + dk
                                d = kc - 4 * qb
                                if d >= 0:
                                    for atX in (atA, atB):
                                        sl = atX[:, dk * 512:dk * 512 + 512]
                                        nc.vector.tensor_mul(
                                            sl, sl,
                                            mask_sb[:, d * 512:d * 512 + 512])
                            for _ in range(2):
                                if filler is not None:
                                    if next(filler, None) is None:
                                        filler = None
                            for dk in range(2):
                                kc = kcg + dk
                                st = (kc == 0)
                                sp = (kc == nkc - 1)
                                nc.tensor.matmul(
                                    avA[:],
                                    lhsT=v_sb[:, kc * 130:kc * 130 + 65],
                                    rhs=atA[:, dk * 512:dk * 512 + 512],
                                    start=st, stop=sp)
                                nc.tensor.matmul(
                                    avB[:],
                                    lhsT=v_sb[:, kc * 130 + 65:
                                              kc * 130 + 130],
                                    rhs=atB[:, dk * 512:dk * 512 + 512],
                                    start=st, stop=sp)
                        # denominators to DRAM gather buf; unnormalized
                        # outputs to concatT
                        for jh, avX in ((0, avA), (1, avB)):
                            row = jh * NQB + qb
                            zrow = norm_pool.tile([1, 512], F32, tag="zrow")
                            nc.vector.tensor_copy(zrow[:], avX[64:65, :])
                            nc.sync.dma_start(zg_d[row:row + 1, :], zrow[:])
                            nc.vector.tensor_copy(
                                concatT[jh * 64:jh * 64 + 64,
                                        c * S + qb * 512:
                                        c * S + qb * 512 + 512],
                                avX[0:64, :])
                    return filler

                def normalize(c, zg_d):
                    zgs = norm_pool.tile([HL, 512], F32, tag="zgs")
                    rzf = norm_pool.tile([HL, 512], F32, tag="rzf")
                    rzb = norm_pool.tile([HL, 512], BF16, tag="rzb")
                    rz_d = dram.tile([HL, 512], BF16, tag="rz_d")
                    nc.sync.dma_start(zgs[:], zg_d[:])
                    nc.vector.reciprocal_approx_fast(rzf[:], zgs[:])
                    nc.vector.tensor_copy(rzb[:], rzf[:])
                    nc.sync.dma_start(rz_d[:], rzb[:])
                    for qb in range(NQB):
                        mt = mult_pool.tile([128, 512], BF16, tag="mult")
                        for jh in range(2):
                            row = jh * NQB + qb
                            nc.sync.dma_start(
                                mt[jh * 64:jh * 64 + 64, :],
                                rz_d[row:row + 1, :].to_broadcast((64, 512)))
                        sl = concatT[:, c * S + qb * 512:
                                     c * S + qb * 512 + 512]
                        nc.vector.tensor_mul(sl, sl, mt[:])
                    # exchange this chunk with the pair peer while later
                    # pairs are still in attention
                    cpart_d = dram.tile([128, S], BF16, tag="cpart")
                    gath_d = dram.tile([256, S], BF16, tag="gath_d")
                    nc.sync.dma_start(cpart_d[:],
                                      concatT[:, c * S:(c + 1) * S])
                    nc.gpsimd.collective_compute(
                        "AllGather",
                        mybir.AluOpType.bypass,
                        replica_groups=[[0, 1], [2, 3], [4, 5], [6, 7]],
                        ins=[cpart_d.opt()],
                        outs=[gath_d.opt()],
                    )
                    nc.sync.dma_start(gathT[c][:, 0:S], gath_d[0:128, :])
                    nc.sync.dma_start(gathT[c][:, S:2 * S],
                                      gath_d[128:256, :])

                # --- main pipeline over head pairs ---
                gen = qkv_steps(0)
                tiles0 = next(gen)
                for _ in gen:
                    pass
                tiles = tiles0
                for c in range(NPAIR):
                    if c + 1 < NPAIR:
                        nxt_gen = qkv_steps(c + 1)
                        nxt_tiles = next(nxt_gen)
                    else:
                        nxt_gen, nxt_tiles = None, None
                    zg_d = dram.tile([HL, 512], F32, tag="zg_d")
                    leftover = attention(c, tiles, nxt_gen, zg_d)
                    if leftover is not None:
                        for _ in leftover:
                            pass
                    normalize(c, zg_d)
                    tiles = nxt_tiles

            # --- output projection over all 8 gathered hd-chunks, for this
            # core's 512 output columns, + full bias ---
            with (
                tc.tile_pool(name="psP", bufs=4, space="PSUM") as psP,
                tc.tile_pool(name="outsb", bufs=4) as outsb,
            ):
                for qc in range(NQC):
                    ps = psP.tile([128, 512], F32, tag="psP")
                    for gc in range(2 * NPAIR):
                        nc.tensor.matmul(
                            ps[:],
                            lhsT=gathT[gc % NPAIR][
                                :, (gc // NPAIR) * S + qc * 128:
                                (gc // NPAIR) * S + qc * 128 + 128],
                            rhs=wproj_sb[:, gc * 512:gc * 512 + 512],
                            start=(gc == 0), stop=False)
                    nc.tensor.matmul(
                        ps[:], lhsT=ones_sb[:], rhs=bias_sb[:],
                        start=False, stop=True)
                    ot = outsb.tile([128, 512], F32, tag="ot")
                    nc.vector.tensor_copy(ot[:], ps[:])
                    nc.sync.dma_start(
                        out[qc * 128:qc * 128 + 128, :], ot[:])

    nc.compile()
    return nc


def _get_nc():
    global _compiled_nc
    if _compiled_nc is None:
        _compiled_nc = _build_nc()
    return _compiled_nc


def _pack_heads(w):
    """[8, 1024, 64] -> [4, 128, 8, 128]: (pair, e128, echunk, jh*64+d)."""
    w = w.reshape(NPAIR, 2, NEC, 128, D)
    w = w.transpose(0, 3, 2, 1, 4)
    return np.ascontiguousarray(w.reshape(NPAIR, 128, NEC, 128))


def _build_masks():
    c2 = np.arange(512)[None, None, :]
    p = np.arange(128)[:, None, None]
    d = np.arange(4)[None, :, None]
    return (c2 >= p + 128 * d).astype(ml_dtypes.bfloat16)


def make_in_maps(x, Wq, Wk, Wv, Wproj, bproj):
    bf = ml_dtypes.bfloat16
    masks_np = _build_masks()
    x = np.asarray(x, dtype=np.float32)
    Wq, Wk, Wv = (np.asarray(a, dtype=np.float32) for a in (Wq, Wk, Wv))
    Wproj = np.asarray(Wproj, dtype=np.float32)
    bproj = np.asarray(bproj, dtype=np.float32)
    in_maps = []
    for c in range(N_CORES):
        b, g = c // 2, c % 2
        hs = slice(g * HL, (g + 1) * HL)
        in_maps.append({
            "xT": np.ascontiguousarray(x[b].T).astype(bf),
            "wqk": np.stack([_pack_heads(Wq[hs]), _pack_heads(Wk[hs])],
                            axis=0).astype(bf),
            "wv": _pack_heads(Wv[hs]).astype(bf),
            "wproj": np.ascontiguousarray(
                Wproj[:, g * 512:(g + 1) * 512]
                .reshape(2 * NPAIR, 128, 512).transpose(1, 0, 2)
            ).astype(bf),
            "biash": bproj[None, g * 512:(g + 1) * 512].astype(bf),
            "masks": masks_np,
        })
    return in_maps


def assemble(results):
    """Each core returns the full S rows for its 512 output columns."""
    out = np.empty((B, S, E), dtype=np.float32)
    for c in range(N_CORES):
        b, g = c // 2, c % 2
        out[b, :, g * 512:(g + 1) * 512] = results[c]["out"]
    return out


def kernel(x, Wq, Wk, Wv, Wproj, bproj):
    nc = _get_nc()
    in_maps = make_in_maps(x, Wq, Wk, Wv, Wproj, bproj)
    res = run_bass_kernel_spmd(nc, in_maps, list(range(N_CORES))).results
    return assemble(res)


# revision 37
# speedup vs baseline: 1.4107x; 1.0148x over previous
"""Multi-head causal attention (B=4, S=2048, E=1024, H=16, D=64) on 8 TRN2
NeuronCores.

Sharding: core c handles batch b = c//2 and head-group g = c%2 (8 heads).
Per core: QKV projections, causal attention, partial output projection over
its 512 input dims of Wproj (+ bproj/2), then a pairwise ReduceScatter sums
the two head-group partials; core 2b returns rows 0:1024 of batch b, core
2b+1 rows 1024:2048.

Layout choices:
 - x is passed pre-transposed per batch: xT [E, S] so it serves directly as
   matmul operands (contraction on partitions).
 - q/k are computed transposed per head-pair: qT/kT [128, S] with rows
   0:64 = head 2c dims, 64:128 = head 2c+1 dims. Scores are computed
   TRANSPOSED (scoresT[k, q]) so that the AV matmul can consume attn with k
   on partitions; the two heads of a pair run concurrently on the PE via
   64-row tile packing.
 - v is stored per k-chunk as [128, 130]: two 65-wide head slots (64 v dims
   + a ones column). The ones column makes the AV matmul emit the softmax
   denominator as output row 64 (no max-subtraction needed: logits are
   O(10), exp is safe in f32).
 - Causal masking: block-granular skipping plus a post-exp multiply by a
   0/1 mask on diagonal blocks only.
"""

import numpy as np
import ml_dtypes

import concourse.bass as bass
import concourse.mybir as mybir
import concourse.bacc as bacc
import concourse.tile as tile
from concourse.bass_utils import run_bass_kernel_spmd

F32 = mybir.dt.float32
BF16 = mybir.dt.bfloat16
EXP = mybir.ActivationFunctionType.Exp

B, S, E, H, D = 4, 2048, 1024, 16, 64
HL = H // 2          # heads per core = 8
NPAIR = HL // 2      # head pairs per core = 4
NEC = E // 128       # e-chunks = 8
NKC = S // 128       # k-chunks = 16
NQB = S // 512       # q blocks = 4
NQC = S // 128       # q chunks = 16
N_CORES = 8

_compiled_nc = None

# diagnostic: stage name per emitted PE instruction (filled during build)
STAGE_OF = {}
_stage = [""]


def _tag_mm(inst):
    try:
        STAGE_OF[inst.ins.name] = _stage[0]
    except Exception:
        pass
    return inst


def _build_nc():
    nc = bacc.Bacc("TRN2", target_bir_lowering=False, debug=False,
                   num_devices=N_CORES)

    xT = nc.dram_tensor("xT", [E, S], BF16, kind="ExternalInput")
    # [qk, pair, 128e-within-chunk, echunk, 128(jh*64+d)]
    wqk = nc.dram_tensor("wqk", [2, NPAIR, 128, NEC, 128], BF16,
                         kind="ExternalInput")
    wv = nc.dram_tensor("wv", [NPAIR, 128, NEC, 128], BF16,
                        kind="ExternalInput")
    # [128(jh*64+d within chunk), global hd-chunk, 512 own e-cols]
    wproj = nc.dram_tensor("wproj", [128, 2 * NPAIR, 512], BF16,
                           kind="ExternalInput")
    biash = nc.dram_tensor("biash", [1, 512], BF16, kind="ExternalInput")
    masks = nc.dram_tensor("masks", [128, 4, 512], BF16, kind="ExternalInput")
    out = nc.dram_tensor("out", [S, 512], F32, kind="ExternalOutput")

    def MM(*a, **k):
        return _tag_mm(nc.tensor.matmul(*a, **k))

    with tile.TileContext(nc) as tc:
        with (
            tc.tile_pool(name="persist", bufs=1) as persist,
            tc.tile_pool(name="dram", bufs=1, space="DRAM") as dram,
        ):
            xT_sb = persist.tile([128, NEC * S], BF16, tag="xT")
            mask_sb = persist.tile([128, 4 * 512], BF16, tag="masks")
            wproj_sb = persist.tile([128, 2 * NPAIR * 512], BF16,
                                    tag="wproj")
            bias_sb = persist.tile([1, 512], BF16, tag="bias")
            ones_sb = persist.tile([1, 128], BF16, tag="ones")
            concatT = persist.tile([128, NPAIR * S], BF16, tag="concatT")
            # gathered concat chunks from both cores of the pair: tile c
            # holds global hd-chunk c (cols 0:S) and chunk 4+c (cols S:2S)
            gathT = [persist.tile([128, 2 * S], BF16, tag=f"gath{c}",
                                  name=f"gath{c}")
                     for c in range(NPAIR)]

            for ec in range(NEC):
                nc.sync.dma_start(xT_sb[:, ec * S:(ec + 1) * S],
                                  xT[ec * 128:(ec + 1) * 128, :])
            nc.sync.dma_start(
                mask_sb[:].rearrange("p (d j) -> p d j", d=4), masks[:])
            nc.sync.dma_start(
                wproj_sb[:].rearrange("p (c e) -> p c e", c=2 * NPAIR),
                wproj[:])
            nc.sync.dma_start(bias_sb[:], biash[:])
            nc.vector.memset(ones_sb[:], 1.0)



            with (
                tc.tile_pool(name="pair", bufs=2) as pair_pool,
                tc.tile_pool(name="attn", bufs=2) as attn_pool,
                tc.tile_pool(name="norm", bufs=2) as norm_pool,
                tc.tile_pool(name="mult", bufs=4) as mult_pool,
                tc.tile_pool(name="psqv", bufs=2, space="PSUM") as psqv,
                tc.tile_pool(name="psS", bufs=1, space="PSUM") as psS,  # 4 banks
                tc.tile_pool(name="psAV", bufs=1, space="PSUM") as psAV,  # 2
            ):
                def qkv_steps(c):
                    """Generator: QKV compute for pair c, one PE group per
                    yield. Produces (qT, kT, v_sb) tiles via closure dict."""
                    res = {}
                    wqk_sb = pair_pool.tile([128, 2 * NEC * 128], BF16,
                                            tag="wqk")
                    wv_sb = pair_pool.tile([128, NEC * 128], BF16, tag="wv")
                    for t in range(2):
                        nc.sync.dma_start(
                            wqk_sb[:, t * 1024:(t + 1) * 1024].rearrange(
                                "p (e j) -> p e j", e=NEC),
                            wqk[t, c])
                    nc.sync.dma_start(
                        wv_sb[:].rearrange("p (e j) -> p e j", e=NEC),
                        wv[c])
                    qT = pair_pool.tile([128, S], BF16, tag="qT")
                    kT = pair_pool.tile([128, S], BF16, tag="kT")
                    v_sb = pair_pool.tile([128, NKC * 130], BF16, tag="v")
                    res["qT"], res["kT"], res["v"] = qT, kT, v_sb
                    yield res
                    _stage[0] = "qk"
                    for t, dst in ((0, qT), (1, kT)):
                        for nb in range(4):
                            ps = psqv.tile([128, 512], F32, tag="psqv")
                            for ec in range(NEC):
                                MM(
                                    ps[:],
                                    lhsT=wqk_sb[:, t * 1024 + ec * 128:
                                                t * 1024 + ec * 128 + 128],
                                    rhs=xT_sb[:, ec * S + nb * 512:
                                              ec * S + nb * 512 + 512],
                                    start=(ec == 0), stop=(ec == NEC - 1))
                            nc.vector.tensor_copy(
                                dst[:, nb * 512:nb * 512 + 512], ps[:])
                            yield res
                    for kc in range(NKC):
                        _stage[0] = "v"
                        ps = psqv.tile([128, 512], F32, tag="psqv")
                        for ec in range(NEC):
                            MM(
                                ps[:, 0:128],
                                lhsT=xT_sb[:, ec * S + kc * 128:
                                           ec * S + kc * 128 + 128],
                                rhs=wv_sb[:, ec * 128:ec * 128 + 128],
                                start=(ec == 0), stop=(ec == NEC - 1))
                        base = kc * 130
                        nc.vector.tensor_copy(
                            v_sb[:, base:base + 130].rearrange(
                                "p (j x) -> p j x", j=2)[:, :, 0:64],
                            ps[:, 0:128].rearrange("p (j d) -> p j d", j=2))
                        nc.vector.memset(v_sb[:, base + 64:base + 65], 1.0)
                        nc.vector.memset(v_sb[:, base + 129:base + 130], 1.0)
                        yield res

                def attention(c, tiles, filler, zg_d):
                    """Attention for pair c. `filler` is a generator whose
                    steps (next pair's QKV groups) are interleaved between
                    scores and AV matmuls to keep the PE busy while ACT
                    does exp."""
                    qT, kT, v_sb = tiles["qT"], tiles["kT"], tiles["v"]
                    for qb in range(NQB):
                        nkc = 4 * qb + 4
                        avA = psAV.tile([65, 512], F32, tag="avA")
                        avB = psAV.tile([65, 512], F32, tag="avB")
                        for kcg in range(0, nkc, 2):
                            sA = psS.tile([128, 1024], F32, tag="sA")
                            sB = psS.tile([128, 1024], F32, tag="sB")
                            _stage[0] = "scores"
                            for dk in range(2):
                                kc = kcg + dk
                                for jh, (sX, o) in enumerate(
                                        ((sA, 0), (sB, 64))):
                                    MM(
                                        sX[:, dk * 512:dk * 512 + 512],
                                        lhsT=kT[o:o + 64,
                                                kc * 128:kc * 128 + 128],
                                        rhs=qT[o:o + 64,
                                               qb * 512:qb * 512 + 512],
                                        start=True, stop=True)
                            atA = attn_pool.tile([128, 1024], BF16,
                                                 tag="atA")
                            atB = attn_pool.tile([128, 1024], BF16,
                                                 tag="atB")
                            nc.scalar.activation(atA[:], sA[:], EXP,
                                                 scale=0.125)
                            nc.scalar.activation(atB[:], sB[:], EXP,
                                                 scale=0.125)
                            for dk in range(2):
                                kc = kcg + dk
                                d = kc - 4 * qb
                                if d >= 0:
                                    for atX in (atA, atB):
                                        sl = atX[:, dk * 512:dk * 512 + 512]
                                        nc.vector.tensor_mul(
                                            sl, sl,
                                            mask_sb[:, d * 512:d * 512 + 512])
                            for _ in range(2):
                                if filler is not None:
                                    if next(filler, None) is None:
                                        filler = None
                            _stage[0] = "av"
                            for dk in range(2):
                                kc = kcg + dk
                                st = (kc == 0)
                                sp = (kc == nkc - 1)
                                MM(
                                    avA[:],
                                    lhsT=v_sb[:, kc * 130:kc * 130 + 65],
                                    rhs=atA[:, dk * 512:dk * 512 + 512],
                                    start=st, stop=sp)
                                MM(
                                    avB[:],
                                    lhsT=v_sb[:, kc * 130 + 65:
                                              kc * 130 + 130],
                                    rhs=atB[:, dk * 512:dk * 512 + 512],
                                    start=st, stop=sp)
                        # denominators to DRAM gather buf; unnormalized
                        # outputs to concatT
                        for jh, avX in ((0, avA), (1, avB)):
                            row = jh * NQB + qb
                            zrow = norm_pool.tile([1, 512], F32, tag="zrow")
                            nc.vector.tensor_copy(zrow[:], avX[64:65, :])
                            nc.sync.dma_start(zg_d[row:row + 1, :], zrow[:])
                            nc.vector.tensor_copy(
                                concatT[jh * 64:jh * 64 + 64,
                                        c * S + qb * 512:
                                        c * S + qb * 512 + 512],
                                avX[0:64, :])
                    return filler

                def normalize(c, zg_d):
                    zgs = norm_pool.tile([HL, 512], F32, tag="zgs")
                    rzf = norm_pool.tile([HL, 512], F32, tag="rzf")
                    rzb = norm_pool.tile([HL, 512], BF16, tag="rzb")
                    rz_d = dram.tile([HL, 512], BF16, tag="rz_d")
                    nc.sync.dma_start(zgs[:], zg_d[:])
                    nc.vector.reciprocal_approx_fast(rzf[:], zgs[:])
                    nc.vector.tensor_copy(rzb[:], rzf[:])
                    nc.sync.dma_start(rz_d[:], rzb[:])
                    for qb in range(NQB):
                        mt = mult_pool.tile([128, 512], BF16, tag="mult")
                        for jh in range(2):
                            row = jh * NQB + qb
                            nc.sync.dma_start(
                                mt[jh * 64:jh * 64 + 64, :],
                                rz_d[row:row + 1, :].to_broadcast((64, 512)))
                        sl = concatT[:, c * S + qb * 512:
                                     c * S + qb * 512 + 512]
                        nc.vector.tensor_mul(sl, sl, mt[:])
                    # exchange this chunk with the pair peer while later
                    # pairs are still in attention
                    cpart_d = dram.tile([128, S], BF16, tag="cpart")
                    gath_d = dram.tile([256, S], BF16, tag="gath_d")
                    nc.sync.dma_start(cpart_d[:],
                                      concatT[:, c * S:(c + 1) * S])
                    nc.gpsimd.collective_compute(
                        "AllGather",
                        mybir.AluOpType.bypass,
                        replica_groups=[[0, 1], [2, 3], [4, 5], [6, 7]],
                        ins=[cpart_d.opt()],
                        outs=[gath_d.opt()],
                    )
                    nc.sync.dma_start(gathT[c][:, 0:S], gath_d[0:128, :])
                    nc.sync.dma_start(gathT[c][:, S:2 * S],
                                      gath_d[128:256, :])

                # --- main pipeline over head pairs ---
                gen = qkv_steps(0)
                tiles0 = next(gen)
                for _ in gen:
                    pass
                tiles = tiles0
                for c in range(NPAIR):
                    if c + 1 < NPAIR:
                        nxt_gen = qkv_steps(c + 1)
                        nxt_tiles = next(nxt_gen)
                    else:
                        nxt_gen, nxt_tiles = None, None
                    zg_d = dram.tile([HL, 512], F32, tag="zg_d")
                    leftover = attention(c, tiles, nxt_gen, zg_d)
                    if leftover is not None:
                        for _ in leftover:
                            pass
                    normalize(c, zg_d)
                    tiles = nxt_tiles

            # --- output projection over all 8 gathered hd-chunks, for this
            # core's 512 output columns, + full bias ---
            with (
                tc.tile_pool(name="psP", bufs=4, space="PSUM") as psP,
                tc.tile_pool(name="outsb", bufs=4) as outsb,
            ):
                for qc in range(NQC):
                    _stage[0] = "proj"
                    ps = psP.tile([128, 512], F32, tag="psP")
                    for gc in range(2 * NPAIR):
                        MM(
                            ps[:],
                            lhsT=gathT[gc % NPAIR][
                                :, (gc // NPAIR) * S + qc * 128:
                                (gc // NPAIR) * S + qc * 128 + 128],
                            rhs=wproj_sb[:, gc * 512:gc * 512 + 512],
                            start=(gc == 0), stop=False)
                    MM(
                        ps[:], lhsT=ones_sb[:], rhs=bias_sb[:],
                        start=False, stop=True)
                    ot = outsb.tile([128, 512], F32, tag="ot")
                    nc.vector.tensor_copy(ot[:], ps[:])
                    nc.sync.dma_start(
                        out[qc * 128:qc * 128 + 128, :], ot[:])

    nc.compile()
    return nc


def _get_nc():
    global _compiled_nc
    if _compiled_nc is None:
        _compiled_nc = _build_nc()
    return _compiled_nc


def _pack_heads(w):
    """[8, 1024, 64] -> [4, 128, 8, 128]: (pair, e128, echunk, jh*64+d)."""
    w = w.reshape(NPAIR, 2, NEC, 128, D)
    w = w.transpose(0, 3, 2, 1, 4)
    return np.ascontiguousarray(w.reshape(NPAIR, 128, NEC, 128))


def _build_masks():
    c2 = np.arange(512)[None, None, :]
    p = np.arange(128)[:, None, None]
    d = np.arange(4)[None, :, None]
    return (c2 >= p + 128 * d).astype(ml_dtypes.bfloat16)


def make_in_maps(x, Wq, Wk, Wv, Wproj, bproj):
    bf = ml_dtypes.bfloat16
    masks_np = _build_masks()
    x = np.asarray(x, dtype=np.float32)
    Wq, Wk, Wv = (np.asarray(a, dtype=np.float32) for a in (Wq, Wk, Wv))
    Wproj = np.asarray(Wproj, dtype=np.float32)
    bproj = np.asarray(bproj, dtype=np.float32)
    in_maps = []
    for c in range(N_CORES):
        b, g = c // 2, c % 2
        hs = slice(g * HL, (g + 1) * HL)
        in_maps.append({
            "xT": np.ascontiguousarray(x[b].T).astype(bf),
            "wqk": np.stack([_pack_heads(Wq[hs]), _pack_heads(Wk[hs])],
                            axis=0).astype(bf),
            "wv": _pack_heads(Wv[hs]).astype(bf),
            "wproj": np.ascontiguousarray(
                Wproj[:, g * 512:(g + 1) * 512]
                .reshape(2 * NPAIR, 128, 512).transpose(1, 0, 2)
            ).astype(bf),
            "biash": bproj[None, g * 512:(g + 1) * 512].astype(bf),
            "masks": masks_np,
        })
    return in_maps


def assemble(results):
    """Each core returns the full S rows for its 512 output columns."""
    out = np.empty((B, S, E), dtype=np.float32)
    for c in range(N_CORES):
        b, g = c // 2, c % 2
        out[b, :, g * 512:(g + 1) * 512] = results[c]["out"]
    return out


def kernel(x, Wq, Wk, Wv, Wproj, bproj):
    nc = _get_nc()
    in_maps = make_in_maps(x, Wq, Wk, Wv, Wproj, bproj)
    res = run_bass_kernel_spmd(nc, in_maps, list(range(N_CORES))).results
    return assemble(res)
